# revision 5
# baseline (speedup 1.0000x reference)
# Trainium2 Bass kernel for nn_DeChunkLayerReference.
#
# Reference semantics (B=4, L=4096, M=2048, D=2048):
#   p = clip(boundary_prob, EPS, 1-EPS) gathered at boundary positions
#       (boundary_mask = every other token -> p[b,i] = p_full[b, 2i])
#   EMA over M steps: h[t] = (1-p[t]) * h[t-1] + p[t] * x[t]   (elementwise in D)
#   out[b, 2i] = out[b, 2i+1] = h[b, i]                        (plug back to L)
#
# Strategy: the EMA is a linear first-order recurrence, computed as blocked
# lower-triangular matmuls on the Tensor engine. Block size K=96: each block's
# input tile holds x rows at partitions [0:kl) and h_in at partition kl
# (kl in {96, 32}, both legal SBUF start partitions); output tile row t<kl is
# y[t] and row kl is h_out = y[kl-1], so the cross-block carry is a
# same-partition [1,512] PSUM->SBUF copy.
#
#   W[s,t] = exp(C[s,t] + lp[s] + mask[s,t]),  C from a matmul of
#   la=log(1-p) against triangular 0/1 matrices.  Y = W.T @ X per block.
#
# Sharding: 8 cores = (batch b in 0..3) x (D half in 0..1). Each core reads an
# (M, 1024) fp32 slice of hidden_states and writes an (L, 1024) output slice
# (each EMA row duplicated to two consecutive output rows).

from contextlib import ExitStack

import numpy as np

import concourse.mybir as mybir
import concourse.tile as tile
from concourse import bacc
from concourse.bass_utils import run_bass_kernel_spmd

EPS = 1e-4
NEG = -1.0e5

B_FULL, L_FULL, M_FULL, D_FULL = 4, 4096, 2048, 2048
DC = D_FULL // 2  # per-core D slice (1024)
N_CORES = 8

f32 = mybir.dt.float32


def build_bass(M=M_FULL, Dc=DC, K=96, x_bufs=5, y_bufs=4, psum_y_bufs=4):
    """Build the per-core Bass program.

    Inputs : p (M,) fp32 (already clipped boundary probs for this batch row)
             x (M, Dc) fp32 (hidden_states slice)
    Output : o (2M, Dc) fp32 (EMA output, each row duplicated twice)
    """
    nfull = M // K
    rem = M - nfull * K
    NB = nfull + (1 if rem else 0)
    assert rem % 32 == 0, "carry slot partition must be 32-aligned"
    KP1 = K + 1
    NCH = (Dc + 511) // 512

    nc = bacc.Bacc("TRN2", target_bir_lowering=False, debug=False)
    p_dram = nc.dram_tensor("p", [M], f32, kind="ExternalInput")
    x_dram = nc.dram_tensor("x", [M, Dc], f32, kind="ExternalInput")
    o_dram = nc.dram_tensor("o", [2 * M, Dc], f32, kind="ExternalOutput")

    Ln = mybir.ActivationFunctionType.Ln
    Exp = mybir.ActivationFunctionType.Exp

    def klen_of(nb):
        return K if nb < nfull else rem

    geoms = sorted({klen_of(nb) for nb in range(NB)}, reverse=True)

    with tile.TileContext(nc) as tc, ExitStack() as ctx:
        const = ctx.enter_context(tc.tile_pool(name="const", bufs=1))
        xpool = ctx.enter_context(tc.tile_pool(name="x", bufs=x_bufs))
        ypool = ctx.enter_context(tc.tile_pool(name="y", bufs=y_bufs))
        wpool = ctx.enter_context(tc.tile_pool(name="w", bufs=3))
        pcs = ctx.enter_context(tc.tile_pool(name="pc", bufs=2, space="PSUM"))
        pys = ctx.enter_context(tc.tile_pool(name="py", bufs=psum_y_bufs, space="PSUM"))

        # --- per-geometry triangular constants ---------------------------
        # For a block of kl steps (inputs s: x[0..kl-1] then h_in at s=kl;
        # outputs t: y[0..kl-1] then h_out=y[kl-1] at t=kl; j(t)=min(t,kl-1)):
        #   amask[r, s] = 1 if (r > s or s == kl) else 0    (r in [0, kl))
        #   bmat [r, t] = 1 if (r <= j(t))                  (single affine)
        #   mmask[s, t] = NEG if (s > t and s < kl) else 0
        amask_g, bmat_g, mmask_g = {}, {}, {}
        for kl in geoms:
            am = const.tile([kl, kl + 1], f32, name=f"amask{kl}")
            nc.vector.memset(am, 1.0)
            nc.gpsimd.affine_select(
                out=am, in_=am, compare_op=mybir.AluOpType.is_gt,
                fill=0.0, base=0, pattern=[[-1, kl + 1]], channel_multiplier=1,
            )
            nc.gpsimd.affine_select(
                out=am, in_=am, compare_op=mybir.AluOpType.is_ge,
                fill=1.0, base=kl - 1, pattern=[[-1, kl + 1]], channel_multiplier=0,
            )
            bm = const.tile([kl, kl + 1], f32, name=f"bmat{kl}")
            nc.vector.memset(bm, 1.0)
            nc.gpsimd.affine_select(
                out=bm, in_=bm, compare_op=mybir.AluOpType.is_ge,
                fill=0.0, base=0, pattern=[[1, kl + 1]], channel_multiplier=-1,
            )
            mm = const.tile([kl + 1, kl + 1], f32, name=f"mmask{kl}")
            nc.vector.memset(mm, 0.0)
            nc.gpsimd.affine_select(
                out=mm, in_=mm, compare_op=mybir.AluOpType.is_ge,
                fill=NEG, base=0, pattern=[[1, kl + 1]], channel_multiplier=-1,
            )
            nc.vector.memset(mm[kl : kl + 1, :], 0.0)
            amask_g[kl], bmat_g[kl], mmask_g[kl] = am, bm, mm

        # --- per-block p-derived tiles -----------------------------------
        # lp_t[s, nb] = log(p[nb*K + s]) for s < klen, 0 at s = klen
        # la_t[r, nb] = log(1 - p[nb*K + r]) for r < klen
        lp_t = const.tile([KP1, NB], f32)
        la_t = const.tile([K, NB], f32)
        p_raw = const.tile([KP1, NB], f32)
        nc.vector.memset(p_raw, 0.5)
        p2d = p_dram.ap()[0 : nfull * K].rearrange("(nb r) -> r nb", r=K)
        nc.sync.dma_start(out=p_raw[0:K, 0:nfull], in_=p2d)
        if rem:
            ptail = p_dram.ap()[nfull * K : M].rearrange("(r one) -> r one", one=1)
            nc.sync.dma_start(out=p_raw[0:rem, nfull : nfull + 1], in_=ptail)
        nc.scalar.activation(out=la_t, in_=p_raw[0:K, :], func=Ln, bias=1.0, scale=-1.0)
        nc.scalar.activation(out=lp_t, in_=p_raw, func=Ln)
        nc.vector.memset(lp_t[K : K + 1, :], 0.0)
        if rem:
            nc.vector.memset(lp_t[rem : rem + 1, nfull : nfull + 1], 0.0)

        # --- main blocked scan -------------------------------------------
        o3 = o_dram.ap().rearrange("(g two) d -> g two d", two=2)

        x_tiles = {}
        x_tiles[0] = xpool.tile([KP1, Dc], f32, tag="xt", name="xt0")
        nc.sync.dma_start(
            out=x_tiles[0][0 : klen_of(0), :],
            in_=x_dram.ap()[0 : klen_of(0), :],
        )
        k0 = klen_of(0)
        nc.vector.memset(x_tiles[0][k0 : k0 + 1, :], 0.0)

        for nb in range(NB):
            kl = klen_of(nb)
            base = nb * K
            if nb + 1 < NB:
                kn = klen_of(nb + 1)
                xn = xpool.tile([KP1, Dc], f32, tag="xt", name=f"xt{nb + 1}")
                nc.sync.dma_start(
                    out=xn[0:kn, :],
                    in_=x_dram.ap()[(nb + 1) * K : (nb + 1) * K + kn, :],
                )
                x_tiles[nb + 1] = xn
            xt = x_tiles.pop(nb)

            # W[s, t] = exp(C[s, t] + mmask[s, t] + lp[s])
            a_t = wpool.tile([K, KP1], f32, tag="a")
            nc.vector.tensor_scalar_mul(
                a_t[0:kl, 0 : kl + 1], amask_g[kl], la_t[0:kl, nb : nb + 1]
            )
            c_ps = pcs.tile([KP1, KP1], f32, tag="cps")
            nc.tensor.matmul(
                c_ps[0 : kl + 1, 0 : kl + 1],
                a_t[0:kl, 0 : kl + 1],
                bmat_g[kl],
                start=True,
                stop=True,
            )
            wr = wpool.tile([KP1, KP1], f32, tag="wr")
            nc.vector.tensor_add(
                wr[0 : kl + 1, 0 : kl + 1],
                c_ps[0 : kl + 1, 0 : kl + 1],
                mmask_g[kl],
            )
            w_t = wpool.tile([KP1, KP1], f32, tag="w")
            nc.scalar.activation(
                out=w_t[0 : kl + 1, 0 : kl + 1],
                in_=wr[0 : kl + 1, 0 : kl + 1],
                func=Exp,
                bias=lp_t[0 : kl + 1, nb : nb + 1],
                scale=1.0,
            )

            # Y = W.T @ X ; rows [0:kl) = outputs, row kl = h_out (next carry)
            y_sb = ypool.tile([KP1, Dc], f32, tag="yt")
            for c in range(NCH):
                c0 = c * 512
                c1 = min(Dc, c0 + 512)
                cw = c1 - c0
                y_ps = pys.tile([KP1, 512], f32, tag="yps")
                nc.tensor.matmul(
                    y_ps[0 : kl + 1, 0:cw],
                    w_t[0 : kl + 1, 0 : kl + 1],
                    xt[0 : kl + 1, c0:c1],
                    start=True,
                    stop=True,
                )
                if nb + 1 < NB:
                    kn = klen_of(nb + 1)
                    nc.vector.tensor_copy(
                        out=x_tiles[nb + 1][kn : kn + 1, c0:c1],
                        in_=y_ps[kl : kl + 1, 0:cw],
                    )
                nc.vector.tensor_copy(out=y_sb[0:kl, c0:c1], in_=y_ps[0:kl, 0:cw])

            nc.sync.dma_start(out=o3[base : base + kl, 0, :], in_=y_sb[0:kl, :])
            nc.sync.dma_start(out=o3[base : base + kl, 1, :], in_=y_sb[0:kl, :])

    nc.compile()
    return nc


_CACHE = {}


def _get_nc():
    if "nc" not in _CACHE:
        _CACHE["nc"] = build_bass()
    return _CACHE["nc"]


def _numpy_fallback(hs, bp, bm, mk):
    """Faithful numpy port of the reference for unexpected mask patterns."""
    B, M, D = hs.shape
    L = bp.shape[1]
    p_full = np.clip(bp.astype(np.float32), EPS, 1.0 - EPS)
    token_idx = np.arange(L)[None, :] + (~bm).astype(np.int32) * L
    seq_sorted = np.argsort(token_idx, axis=1, kind="stable")
    p = np.take_along_axis(p_full, seq_sorted[:, :M], axis=1)
    p = np.clip(p, EPS, 1.0 - EPS)
    h = np.zeros((B, D), np.float32)
    y = np.empty((B, M, D), np.float32)
    for t in range(M):
        h = (1.0 - p[:, t])[:, None] * h + p[:, t][:, None] * hs[:, t, :]
        y[:, t, :] = h
    plug_back = np.cumsum(bm.astype(np.int32), axis=1) - 1
    plug_back = np.clip(plug_back, 0, M - 1)
    out = np.take_along_axis(y, plug_back[..., None], axis=1)
    return out.astype(np.float32)


def _make_in_maps(hs, p):
    in_maps = []
    for core in range(N_CORES):
        b, h = core // 2, core % 2
        in_maps.append(
            {
                "p": np.ascontiguousarray(p[b]),
                "x": np.ascontiguousarray(hs[b, :, h * DC : (h + 1) * DC]),
            }
        )
    return in_maps


def _assemble(results):
    out = np.empty((B_FULL, L_FULL, D_FULL), np.float32)
    for core in range(N_CORES):
        b, h = core // 2, core % 2
        out[b, :, h * DC : (h + 1) * DC] = results[core]["o"]
    return out


def kernel(hidden_states, boundary_prob, boundary_mask, mask, **run_kwargs):
    hs = np.asarray(hidden_states, dtype=np.float32)
    bp = np.asarray(boundary_prob, dtype=np.float32)
    bm = np.asarray(boundary_mask, dtype=bool)
    mk = np.asarray(mask, dtype=bool)

    expected_mask = np.arange(bp.shape[1]) % 2 == 0
    if (
        hs.shape != (B_FULL, M_FULL, D_FULL)
        or bp.shape != (B_FULL, L_FULL)
        or not bool((bm == expected_mask[None, :]).all())
    ):
        return _numpy_fallback(hs, bp, bm, mk)

    p = np.clip(bp, EPS, 1.0 - EPS)[:, ::2].astype(np.float32)
    p = np.clip(p, EPS, 1.0 - EPS)
    res = run_bass_kernel_spmd(
        _get_nc(), _make_in_maps(hs, p), core_ids=list(range(N_CORES)), **run_kwargs
    )
    out = _assemble(res.results)
    if run_kwargs:
        _CACHE["last_results"] = res
    return out
